# revision 1
# baseline (speedup 1.0000x reference)
"""Trainium2 Bass kernel for nn_CausalAttention_5815385719336.

Dual-softmax attention: out = softmax(-QK^T/8) V Wo^T (+bias folds),
out_comp = softmax(+QK^T/8) V Wo^T.  B=2, S=2048, D=1024, H=16, DK=64.

Sharding (8 cores): Megatron-style head parallel.  Core c owns heads
(2c, 2c+1) = output dims [128c, 128c+128) of the QKV projections.  Each
core computes its head slice of Q/K/V for both batches, the full [S,S]
attention for its 4 (b, head) units (both softmax branches), and a
partial output projection o_slice @ Wo_slice^T.  The host sums the 8
partial outputs and adds the bias fold (bv @ Wo^T + bo).

On-device dataflow is fully "transposed": the host ships x^T (and W^T
slices) so every matmul contracts along partitions with zero on-device
transposes.  Scores are built as scores^T [k, q]; exp runs on the
scalar engine straight out of PSUM; P^T @ V needs no transpose because
P^T is exactly what the PV matmul wants as its moving operand.  The
softmax denominator comes for free from a ones-column appended to V
(one extra PSUM row per head), is broadcast across partitions on
GPSIMD, reciprocated with the fast custom-DVE op, and folded into the
oT tiles before the output projection.
"""

import numpy as np
import ml_dtypes

B, S, D, H, DK = 2, 2048, 1024, 16, 64
NCORES = 8
HPC = H // NCORES          # heads per core = 2
DSL = HPC * DK             # d-slice per core = 128
P = 128
BF16 = ml_dtypes.bfloat16

_compiled = {}


def _install_drain_split():
    """walrus in this container rejects >1 sync wait on the Tile tail
    Drain; split extra waits into standalone wait_ge instructions."""
    import concourse.tile as tile
    from concourse.vector_clock import ScopedClock

    if getattr(tile.TileContext, "_drain_split_installed", False):
        return

    def _drain_and_barrier(self, tick_clock, wait_clock):
        nc = self.nc
        drain_inst = nc.sync.drain()
        wait_clock.add_sem_waits(
            drain_inst.ins, ScopedClock({None: tick_clock.global_clock})
        )
        si = drain_inst.ins.sync_info
        if si is not None and si.on_wait and len(si.on_wait) > 1:
            waits = list(si.on_wait)
            handles = {h.num: h for h in self.sems.allocated().values()}
            si.on_wait = waits[:1]
            for w in waits[1:]:
                assert w.wait_mode == "sem-ge-imm", w.wait_mode
                nc.sync.wait_ge(handles[w.id], w.wait_value)
        nc.all_engine_barrier()
        popped = nc._tile_sem_poison_stack.pop()
        assert popped is self._sem_poison
        nc.clear_and_free_semaphores(list(self.sems.allocated().values()))
        nc.all_engine_barrier()

    tile.TileContext._drain_and_barrier = _drain_and_barrier
    tile.TileContext._drain_split_installed = True


def _split_sync_waits(nc, max_waits=1):
    """walrus in this container has a small per-instruction sync-wait
    capacity.  Hoist excess waits onto standalone EventSemaphore
    instructions inserted just before the owner on the same engine —
    program order within an engine keeps the semantics identical."""
    from concourse import mybir

    n = 0
    for bb in nc.main_func.blocks:
        out = []
        for ins in bb.instructions:
            si = ins.sync_info
            if si is not None and si.on_wait and len(si.on_wait) > max_waits:
                waits = list(si.on_wait)
                for w in waits[:-max_waits]:
                    wi = mybir.InstEventSemaphore(name=f"W-split-{n}", ins=[], outs=[])
                    n += 1
                    wi.engine = ins.engine
                    wi.sync_info = mybir.SyncInfo(on_wait=[w], on_update=[])
                    out.append(wi)
                si.on_wait = waits[-max_waits:]
            out.append(ins)
        if n:
            bb.instructions = out


def _build():
    import concourse.bass as bass
    import concourse.tile as tile
    from concourse import mybir

    _install_drain_split()

    f32 = mybir.dt.float32
    bf16 = mybir.dt.bfloat16
    Exp = mybir.ActivationFunctionType.Exp
    Log = mybir.ActivationFunctionType.Ln
    NT = B * S                      # 4096 tokens
    ET = D // P                     # 8 e-tiles

    nc = bass.Bass()
    xt_d = nc.declare_dram_parameter("xt", [P, ET, NT], bf16, isOutput=False)
    wq_d = nc.declare_dram_parameter("wq", [P, ET, DSL], bf16, isOutput=False)
    wk_d = nc.declare_dram_parameter("wk", [P, ET, DSL], bf16, isOutput=False)
    wv_d = nc.declare_dram_parameter("wv", [P, ET, DSL], bf16, isOutput=False)
    wo_d = nc.declare_dram_parameter("wo", [P, D], bf16, isOutput=False)
    bq_d = nc.declare_dram_parameter("bq", [P, 1], f32, isOutput=False)
    bk_d = nc.declare_dram_parameter("bk", [P, 1], f32, isOutput=False)
    out_d = nc.declare_dram_parameter("out", [2, B, S, D], bf16, isOutput=True)

    KT = S // P                     # 16 k-tiles per batch
    TT = S // P                     # 16 token-tiles per batch
    QC = 2                          # q chunks per batch
    QW = S // QC                    # 1024

    with tile.TileContext(nc) as tc:
        with (
            tc.tile_pool(name="singles", bufs=1) as singles,
            tc.tile_pool(name="xst", bufs=2) as xst,
            tc.tile_pool(name="perb", bufs=2) as perb,
            tc.tile_pool(name="stash", bufs=2) as stash,
            tc.tile_pool(name="expp", bufs=3) as expp,
            tc.tile_pool(name="otsp", bufs=2) as otsp,
            tc.tile_pool(name="normp", bufs=3) as normp,
            tc.tile_pool(name="outp", bufs=3) as outp,
            # 8 PSUM banks total: ps_big 2x[128,1024] = 4, ps_acc 2x = 4.
            # Everything besides the two PV accumulators (scores, denom
            # broadcast, projections, outproj) shares ps_big.
            tc.tile_pool(name="ps_big", bufs=2, space="PSUM") as ps_big,
            tc.tile_pool(name="ps_acc", bufs=2, space="PSUM") as ps_acc,
        ):
            wq = singles.tile([P, ET, DSL], bf16)
            nc.sync.dma_start(wq[:], wq_d[:])
            wk = singles.tile([P, ET, DSL], bf16)
            nc.sync.dma_start(wk[:], wk_d[:])
            wv = singles.tile([P, ET, DSL], bf16)
            nc.sync.dma_start(wv[:], wv_d[:])
            wo = singles.tile([P, D], bf16)
            nc.sync.dma_start(wo[:], wo_d[:])
            bq = singles.tile([P, 1], f32)
            nc.sync.dma_start(bq[:], bq_d[:])
            bk = singles.tile([P, 1], f32)
            nc.sync.dma_start(bk[:], bk_d[:])
            ones_sb = singles.tile([P, 64], bf16)
            nc.vector.memset(ones_sb[:], 1.0)

            # ---------- helpers ----------
            # background queue of deferred PE-work closures, drained at
            # attention kt-boundaries to keep the PE dense (HAM-warm) while
            # ACT paces the kernel
            bg_queue = []

            def drain_bg(n=1):
                for _ in range(n):
                    if not bg_queue:
                        return
                    bg_queue.pop(0)()

            # one pending negative-branch PV accumulation, interleaved into
            # the next unit's kt loop
            pending = {}

            def emit_pending_mms(k0, k1):
                if not pending:
                    return
                exn, accn, vtp, vlo, vhi = (pending[k] for k in
                                            ("exn", "acc", "vt", "vlo", "vhi"))
                for kt in range(k0, k1):
                    for fh in range(2):
                        nc.tensor.matmul(
                            accn[0:65, fh * 512 : (fh + 1) * 512],
                            vtp[:, kt, vlo:vhi],
                            exn[:, kt, fh * 512 : (fh + 1) * 512],
                            start=(kt == 0),
                            stop=(kt == KT - 1),
                        )

            def normalize(acc, oTs_br, h, q0, name):
                """1/denom = exp(-Ln(denom)); broadcast via ones-matmul;
                multiply the unnormalized oT rows into their oTs slice."""
                hp = 64 * h
                lnd = normp.tile([P, QW], f32, tag="lnd", name=f"lnd{name}")
                nc.scalar.activation(lnd[64:65, :], acc[64:65, :], Log)
                rcp = normp.tile([P, QW], bf16, tag="rcp", name=f"rcp{name}")
                nc.scalar.activation(rcp[64:65, :], lnd[64:65, :], Exp,
                                     scale=-1.0)
                oTu = normp.tile([P, QW], bf16, tag="oTu", name=f"oTu{name}")
                nc.vector.tensor_copy(oTu[0:64, :], acc[0:64, :])
                if h == 1:
                    # shift head-1 rows to partitions 64:128 so the output
                    # projection contracts both heads in one matmul
                    oTu2 = normp.tile([P, QW], bf16, tag="oTu2",
                                      name=f"oTu2{name}")
                    nc.sync.dma_start(oTu2[64:128, :], oTu[0:64, :])
                    oTu = oTu2
                bc = ps_acc.tile([P, QW], f32, tag="acc", name=f"bc{name}")
                bc_ap = bc[0:64] if h == 0 else bc[64:128]
                for fh in range(2):
                    nc.tensor.matmul(
                        bc_ap[:, fh * 512 : (fh + 1) * 512],
                        ones_sb[64:65, :],
                        rcp[64:65, fh * 512 : (fh + 1) * 512],
                        start=True,
                        stop=True,
                    )
                nc.vector.tensor_mul(
                    oTs_br[hp : hp + 64, q0 : q0 + QW],
                    oTu[hp : hp + 64, :],
                    bc_ap,
                )

            def finish_pending():
                if not pending:
                    return
                normalize(pending["acc"], pending["oTs_br"], pending["h"],
                          pending["q0"], pending["name"] + "n")
                pending.clear()

            def queue_projections(b):
                """Queue Q/K/V projection chunks for batch b (streamed x)."""
                t0 = b * S
                qT = perb.tile([P, S], bf16, tag="qT", name=f"qT_{b}")
                kT = perb.tile([P, S], bf16, tag="kT", name=f"kT_{b}")
                vt = perb.tile([P, TT, 130], bf16, tag="vt", name=f"vt_{b}")
                nc.vector.memset(vt[:, :, 64], 1.0)
                nc.vector.memset(vt[:, :, 129], 1.0)
                cell = {}

                def load_chunk(qc):
                    def go():
                        xtile = xst.tile([P, ET, 512], bf16, tag="xtile",
                                         name=f"xt_{b}_{qc}")
                        nc.sync.dma_start(
                            xtile[:],
                            xt_d[:, :, t0 + qc * 512 : t0 + (qc + 1) * 512],
                        )
                        cell[qc] = xtile
                    return go

                def qk_chunk(qc, w_t, bias_t, dst):
                    def go():
                        xtile = cell[qc]
                        ps = ps_big.tile([P, 512], f32, tag="big",
                                         name=f"pj_{b}_{qc}_{id(w_t)}")
                        for et in range(ET):
                            nc.tensor.matmul(
                                ps, w_t[:, et, :], xtile[:, et, :],
                                start=(et == 0), stop=(et == ET - 1),
                            )
                        nc.vector.tensor_scalar_add(
                            dst[:, qc * 512 : (qc + 1) * 512], ps, bias_t
                        )
                    return go

                def v_chunk(qc, vtt):
                    def go():
                        xtile = cell[qc]
                        tt = qc * 4 + vtt
                        pv = ps_big.tile([P, DSL], f32, tag="big",
                                         name=f"pv_{b}_{tt}")
                        for et in range(ET):
                            nc.tensor.matmul(
                                pv, xtile[:, et, vtt * P : (vtt + 1) * P],
                                wv[:, et, :],
                                start=(et == 0), stop=(et == ET - 1),
                            )
                        nc.vector.tensor_copy(vt[:, tt, 0:64], pv[:, 0:64])
                        nc.vector.tensor_copy(vt[:, tt, 65:129],
                                              pv[:, 64:128])
                    return go

                chunks = []
                for qc in range(4):
                    chunks.append(load_chunk(qc))
                    chunks.append(qk_chunk(qc, wq, bq, qT))
                    chunks.append(qk_chunk(qc, wk, bk, kT))
                    for vtt in range(4):
                        chunks.append(v_chunk(qc, vtt))
                return qT, kT, vt, chunks

            def outproj_chunks(b, oTs):
                """Output projection chunk closures for batch b."""
                chunks = []

                def one(br, tt):
                    def go():
                        po = ps_big.tile([P, D], f32, tag="big",
                                         name=f"po_{b}_{br}_{tt}")
                        for oc in range(2):
                            nc.tensor.matmul(
                                po[:, oc * 512 : (oc + 1) * 512],
                                oTs[br][:, tt * P : (tt + 1) * P],
                                wo[:, oc * 512 : (oc + 1) * 512],
                                start=True,
                                stop=True,
                            )
                        ob = outp.tile([P, D], bf16, tag="ob")
                        nc.vector.tensor_copy(ob[:], po[:])
                        nc.sync.dma_start(
                            out_d[br, b, tt * P : (tt + 1) * P, :], ob[:]
                        )
                    return go

                for br in range(2):
                    for tt in range(TT):
                        chunks.append(one(br, tt))
                return chunks

            def attention(b, qT, kT, vt, oTs):
                for h in range(HPC):
                    hp = 64 * h
                    vlo, vhi = (0, 65) if h == 0 else (65, 130)
                    for qc in range(QC):
                        q0 = qc * QW
                        name = f"_{b}_{h}_{qc}"
                        exn = stash.tile([P, KT, QW], bf16, tag="exn",
                                         name=f"exn{name}")
                        acc = ps_acc.tile([P, QW], f32, tag="acc",
                                          name=f"accp{name}")
                        for kt in range(KT):
                            sc = ps_big.tile([P, QW], f32, tag="big",
                                             name=f"sc{name}_{kt}")
                            for fh in range(2):
                                nc.tensor.matmul(
                                    sc[:, fh * 512 : (fh + 1) * 512],
                                    kT[hp : hp + 64, kt * P : (kt + 1) * P],
                                    qT[hp : hp + 64,
                                       q0 + fh * 512 : q0 + (fh + 1) * 512],
                                    start=True,
                                    stop=True,
                                )
                            ex = expp.tile([P, QW], bf16, tag="ex")
                            nc.scalar.activation(ex, sc, Exp, scale=-0.125)
                            nc.scalar.activation(exn[:, kt, :], sc, Exp,
                                                 scale=0.125)
                            for fh in range(2):
                                nc.tensor.matmul(
                                    acc[0:65, fh * 512 : (fh + 1) * 512],
                                    vt[:, kt, vlo:vhi],
                                    ex[:, fh * 512 : (fh + 1) * 512],
                                    start=(kt == 0),
                                    stop=(kt == KT - 1),
                                )
                            emit_pending_mms(kt, kt + 1)
                            if kt % 2 == 1:
                                drain_bg(1)
                        finish_pending()
                        normalize(acc, oTs[0], h, q0, name + "p")
                        accn = ps_acc.tile([P, QW], f32, tag="acc",
                                           name=f"accn{name}")
                        pending.update(exn=exn, acc=accn, vt=vt, vlo=vlo,
                                       vhi=vhi, oTs_br=oTs[1], h=h, q0=q0,
                                       name=name)

            # ---------- emission ----------
            qT0, kT0, vt0, pchunks0 = queue_projections(0)
            for ch in pchunks0:       # batch-0 projections run up front
                ch()
            qT1, kT1, vt1, pchunks1 = queue_projections(1)
            bg_queue.extend(pchunks1)  # batch-1 projections hide in b0 attn

            oTs0 = [otsp.tile([P, S], bf16, tag=f"oTs{br}", name=f"oTs{br}_0")
                    for br in range(2)]
            oTs1 = [otsp.tile([P, S], bf16, tag=f"oTs{br}", name=f"oTs{br}_1")
                    for br in range(2)]

            attention(0, qT0, kT0, vt0, oTs0)
            bg_queue.extend(outproj_chunks(0, oTs0))  # hide in b1 attn
            attention(1, qT1, kT1, vt1, oTs1)

            # tail: alternate the last pending PV with batch-1 br0
            # output-projection chunks; br1 chunks only after the pending
            # norm has written its oTs[1] slice (emission order is
            # semantics under Tile's tracing)
            op1 = outproj_chunks(1, oTs1)
            bg_queue.extend(op1[:TT])
            for kt in range(KT):
                emit_pending_mms(kt, kt + 1)
                drain_bg(1)
            finish_pending()
            bg_queue.extend(op1[TT:])
            drain_bg(len(bg_queue))
    _split_sync_waits(nc)
    return nc




def _get_nc():
    if "nc" not in _compiled:
        _compiled["nc"] = _build()
    return _compiled["nc"]


def _prep_in_maps(x, Wq, bq, Wk, bk, Wv, bv, Wo, bo):
    ET = D // P
    xf = np.ascontiguousarray(x.reshape(B * S, D))
    # x^T tiled: [p, et, token], e = et*128 + p
    xt = np.ascontiguousarray(
        xf.T.reshape(ET, P, B * S).transpose(1, 0, 2)
    ).astype(BF16)
    in_maps = []
    for c in range(NCORES):
        sl = slice(DSL * c, DSL * (c + 1))
        wqt = np.ascontiguousarray(
            Wq[sl].T.reshape(ET, P, DSL).transpose(1, 0, 2)
        ).astype(BF16)
        wkt = np.ascontiguousarray(
            Wk[sl].T.reshape(ET, P, DSL).transpose(1, 0, 2)
        ).astype(BF16)
        wvt = np.ascontiguousarray(
            Wv[sl].T.reshape(ET, P, DSL).transpose(1, 0, 2)
        ).astype(BF16)
        wot = np.ascontiguousarray(Wo[:, sl].T).astype(BF16)
        in_maps.append(
            {
                "xt": xt,
                "wq": wqt,
                "wk": wkt,
                "wv": wvt,
                "wo": wot,
                "bq": np.ascontiguousarray(bq[sl].reshape(P, 1)).astype(np.float32),
                "bk": np.ascontiguousarray(bk[sl].reshape(P, 1)).astype(np.float32),
            }
        )
    return in_maps


def kernel(x, Wq, bq, Wk, bk, Wv, bv, Wo, bo, _trace=False, _tmpdir=None):
    from concourse.bass_utils import run_bass_kernel_spmd

    x, Wq, bq, Wk, bk, Wv, bv, Wo, bo = (
        np.asarray(a, dtype=np.float32)
        for a in (x, Wq, bq, Wk, bk, Wv, bv, Wo, bo)
    )
    nc = _get_nc()
    in_maps = _prep_in_maps(x, Wq, bq, Wk, bk, Wv, bv, Wo, bo)
    res = run_bass_kernel_spmd(
        nc, in_maps, core_ids=list(range(NCORES)), trace=_trace, tmpdir=_tmpdir
    )
    total = np.zeros((2, B, S, D), np.float32)
    for c in range(NCORES):
        total += np.asarray(res.results[c]["out"], dtype=np.float32)
    const_vec = (bv @ Wo.T + bo).astype(np.float32)
    out = total[0] + const_vec
    out_comp = total[1] + const_vec
    if _trace:
        kernel._last_result = res
    return (out, out_comp)



# revision 9
# speedup vs baseline: 1.2952x; 1.2952x over previous
"""Trainium2 Bass kernel for nn_CausalAttention_5815385719336.

Dual-softmax attention: out = softmax(-QK^T/8) V Wo^T (+bias folds),
out_comp = softmax(+QK^T/8) V Wo^T.  B=2, S=2048, D=1024, H=16, DK=64.

Sharding (8 cores): Megatron-style head parallel.  Core c owns heads
(2c, 2c+1) = output dims [128c, 128c+128) of the QKV projections.  Each
core computes its head slice of Q/K/V for both batches, the full [S,S]
attention for its 4 (b, head) units (both softmax branches), and a
partial output projection o_slice @ Wo_slice^T.  The host sums the 8
partial outputs and adds the bias fold (bv @ Wo^T + bo).

On-device dataflow is fully "transposed": the host ships x^T (and W^T
slices) so every matmul contracts along partitions with zero on-device
transposes.  Scores are built as scores^T [k, q]; the negative-branch
exp runs on the scalar engine straight out of PSUM and the positive
branch is its reciprocal on the DVE (custom RECIPROCAL_APPROX_FAST op),
splitting the 33.6M-element exp workload across two engines.  P^T @ V
needs no transpose because P^T is exactly what the PV matmul wants as
its moving operand.  The softmax denominator comes for free from a
ones-column appended to V (one extra PSUM row per head), is
reciprocated on the DVE and broadcast across partitions by a
ones-matmul on the PE; PSUM->SBUF output casts run on the scalar
engine (closer to PSUM) to keep the DVE free for the reciprocals.
"""

import numpy as np
import ml_dtypes

B, S, D, H, DK = 2, 2048, 1024, 16, 64
NCORES = 8
HPC = H // NCORES          # heads per core = 2
DSL = HPC * DK             # d-slice per core = 128
P = 128
BF16 = ml_dtypes.bfloat16

_compiled = {}


def _install_drain_split():
    """walrus in this container rejects >1 sync wait on the Tile tail
    Drain; split extra waits into standalone wait_ge instructions."""
    import concourse.tile as tile
    from concourse.vector_clock import ScopedClock

    if getattr(tile.TileContext, "_drain_split_installed", False):
        return

    def _drain_and_barrier(self, tick_clock, wait_clock):
        nc = self.nc
        drain_inst = nc.sync.drain()
        wait_clock.add_sem_waits(
            drain_inst.ins, ScopedClock({None: tick_clock.global_clock})
        )
        si = drain_inst.ins.sync_info
        if si is not None and si.on_wait and len(si.on_wait) > 1:
            waits = list(si.on_wait)
            handles = {h.num: h for h in self.sems.allocated().values()}
            si.on_wait = waits[:1]
            for w in waits[1:]:
                assert w.wait_mode == "sem-ge-imm", w.wait_mode
                nc.sync.wait_ge(handles[w.id], w.wait_value)
        nc.all_engine_barrier()
        popped = nc._tile_sem_poison_stack.pop()
        assert popped is self._sem_poison
        nc.clear_and_free_semaphores(list(self.sems.allocated().values()))
        nc.all_engine_barrier()

    tile.TileContext._drain_and_barrier = _drain_and_barrier
    tile.TileContext._drain_split_installed = True


def _split_sync_waits(nc, max_waits=1):
    """walrus in this container has a small per-instruction sync-wait
    capacity.  Hoist excess waits onto standalone EventSemaphore
    instructions inserted just before the owner on the same engine —
    program order within an engine keeps the semantics identical."""
    from concourse import mybir

    n = 0
    for bb in nc.main_func.blocks:
        out = []
        for ins in bb.instructions:
            si = ins.sync_info
            if si is not None and si.on_wait and len(si.on_wait) > max_waits:
                waits = list(si.on_wait)
                for w in waits[:-max_waits]:
                    wi = mybir.InstEventSemaphore(name=f"W-split-{n}", ins=[], outs=[])
                    n += 1
                    wi.engine = ins.engine
                    wi.sync_info = mybir.SyncInfo(on_wait=[w], on_update=[])
                    out.append(wi)
                si.on_wait = waits[-max_waits:]
            out.append(ins)
        if n:
            bb.instructions = out


def _build():
    import concourse.bass as bass
    import concourse.tile as tile
    from concourse import mybir

    _install_drain_split()

    from concourse.dve_ops import (
        RECIP_APPROX_FAST_CONSTS,
        RECIPROCAL_APPROX_FAST,
    )

    f32 = mybir.dt.float32
    bf16 = mybir.dt.bfloat16
    Exp = mybir.ActivationFunctionType.Exp
    NT = B * S                      # 4096 tokens
    ET = D // P                     # 8 e-tiles
    RC = RECIP_APPROX_FAST_CONSTS

    nc = bass.Bass()
    xt_d = nc.declare_dram_parameter("xt", [P, ET, NT], bf16, isOutput=False)
    wq_d = nc.declare_dram_parameter("wq", [P, ET, DSL], bf16, isOutput=False)
    wk_d = nc.declare_dram_parameter("wk", [P, ET, DSL], bf16, isOutput=False)
    wv_d = nc.declare_dram_parameter("wv", [P, ET, DSL], bf16, isOutput=False)
    wo_d = nc.declare_dram_parameter("wo", [P, D], bf16, isOutput=False)
    bq_d = nc.declare_dram_parameter("bq", [P, 1], f32, isOutput=False)
    bk_d = nc.declare_dram_parameter("bk", [P, 1], f32, isOutput=False)
    out_d = nc.declare_dram_parameter("out", [2, B, S, D], bf16, isOutput=True)

    KT = S // P                     # 16 k-tiles per batch
    TT = S // P                     # 16 token-tiles per batch
    QC = 2                          # q chunks per batch
    QW = S // QC                    # 1024

    with tile.TileContext(nc) as tc:
        with (
            tc.tile_pool(name="singles", bufs=1) as singles,
            tc.tile_pool(name="xst", bufs=2) as xst,
            tc.tile_pool(name="perb", bufs=2) as perb,
            tc.tile_pool(name="stash", bufs=2) as stash,
            tc.tile_pool(name="expp", bufs=3) as expp,
            tc.tile_pool(name="otsp", bufs=2) as otsp,
            tc.tile_pool(name="normp", bufs=3) as normp,
            tc.tile_pool(name="outp", bufs=3) as outp,
            # 8 PSUM banks total: ps_big 2x[128,1024] = 4, ps_acc 2x = 4.
            # Everything besides the two PV accumulators (scores, denom
            # broadcast, projections, outproj) shares ps_big.
            tc.tile_pool(name="ps_big", bufs=2, space="PSUM") as ps_big,
            tc.tile_pool(name="ps_acc", bufs=2, space="PSUM") as ps_acc,
        ):
            wq = singles.tile([P, ET, DSL], bf16)
            nc.sync.dma_start(wq[:], wq_d[:])
            wk = singles.tile([P, ET, DSL], bf16)
            nc.sync.dma_start(wk[:], wk_d[:])
            wv = singles.tile([P, ET, DSL], bf16)
            nc.sync.dma_start(wv[:], wv_d[:])
            wo = singles.tile([P, D], bf16)
            nc.sync.dma_start(wo[:], wo_d[:])
            bq = singles.tile([P, 1], f32)
            nc.sync.dma_start(bq[:], bq_d[:])
            bk = singles.tile([P, 1], f32)
            nc.sync.dma_start(bk[:], bk_d[:])
            ones_sb = singles.tile([P, 64], bf16)
            nc.vector.memset(ones_sb[:], 1.0)

            def recip(out_ap, in_ap):
                """out = 1/in on the DVE (custom op, ~51 ULP in fp32; the
                bf16 write rounds it to bf16 anyway).  Frees the ACT engine
                from the positive-branch exp: exp(+s) = 1/exp(-s)."""
                nc.vector._custom_dve(
                    RECIPROCAL_APPROX_FAST, out=out_ap, in0=in_ap,
                    s0=RC["s0"], s1=RC["s1"], imm2=RC["imm2"],
                )

            # ---------- helpers ----------
            # background queue of deferred PE-work closures, drained at
            # attention kt-boundaries to keep the PE dense (HAM-warm) while
            # ACT paces the kernel
            bg_queue = []

            def drain_bg(n=1):
                for _ in range(n):
                    if not bg_queue:
                        return
                    bg_queue.pop(0)()

            # one pending negative-branch PV accumulation, interleaved into
            # the next unit's kt loop
            pending = {}

            def emit_pending_mms(k0, k1):
                if not pending:
                    return
                exn, accn, vtp, vlo, vhi = (pending[k] for k in
                                            ("exn", "acc", "vt", "vlo", "vhi"))
                for kt in range(k0, k1):
                    for fh in range(2):
                        nc.tensor.matmul(
                            accn[0:65, fh * 512 : (fh + 1) * 512],
                            vtp[:, kt, vlo:vhi],
                            exn[:, kt, fh * 512 : (fh + 1) * 512],
                            start=(kt == 0),
                            stop=(kt == KT - 1),
                        )

            def normalize(acc, oTs_br, h, q0, name):
                """1/denom via the DVE recip; broadcast via ones-matmul;
                multiply the unnormalized oT rows into their oTs slice."""
                hp = 64 * h
                rcp = normp.tile([P, QW], bf16, tag="rcp", name=f"rcp{name}")
                # the custom-DVE op only functions at base_partition 0, so
                # reciprocate rows 0..64 (cost is free-dim-bound; the junk
                # recips of the o-rows in 0:64 are never read — only the
                # denominator row 64 is consumed by the broadcast matmul)
                recip(rcp[0:65, :], acc[0:65, :])
                oTu = normp.tile([P, QW], bf16, tag="oTu", name=f"oTu{name}")
                nc.scalar.copy(oTu[0:64, :], acc[0:64, :])
                if h == 1:
                    # shift head-1 rows to partitions 64:128 so the output
                    # projection contracts both heads in one matmul
                    oTu2 = normp.tile([P, QW], bf16, tag="oTu2",
                                      name=f"oTu2{name}")
                    nc.sync.dma_start(oTu2[64:128, :], oTu[0:64, :])
                    oTu = oTu2
                bc = ps_acc.tile([P, QW], f32, tag="acc", name=f"bc{name}")
                bc_ap = bc[0:64] if h == 0 else bc[64:128]
                for fh in range(2):
                    nc.tensor.matmul(
                        bc_ap[:, fh * 512 : (fh + 1) * 512],
                        ones_sb[64:65, :],
                        rcp[64:65, fh * 512 : (fh + 1) * 512],
                        start=True,
                        stop=True,
                    )
                nc.vector.tensor_mul(
                    oTs_br[hp : hp + 64, q0 : q0 + QW],
                    oTu[hp : hp + 64, :],
                    bc_ap,
                )

            def finish_pending():
                if not pending:
                    return
                normalize(pending["acc"], pending["oTs_br"], pending["h"],
                          pending["q0"], pending["name"] + "n")
                pending.clear()

            def queue_projections(b):
                """Queue Q/K/V projection chunks for batch b (streamed x)."""
                t0 = b * S
                qT = perb.tile([P, S], bf16, tag="qT", name=f"qT_{b}")
                kT = perb.tile([P, S], bf16, tag="kT", name=f"kT_{b}")
                vt = perb.tile([P, TT, 130], bf16, tag="vt", name=f"vt_{b}")
                nc.vector.memset(vt[:, :, 64], 1.0)
                nc.vector.memset(vt[:, :, 129], 1.0)
                cell = {}

                def load_chunk(qc):
                    def go():
                        xtile = xst.tile([P, ET, 512], bf16, tag="xtile",
                                         name=f"xt_{b}_{qc}")
                        nc.sync.dma_start(
                            xtile[:],
                            xt_d[:, :, t0 + qc * 512 : t0 + (qc + 1) * 512],
                        )
                        cell[qc] = xtile
                    return go

                def qk_chunk(qc, w_t, bias_t, dst):
                    def go():
                        xtile = cell[qc]
                        ps = ps_big.tile([P, 512], f32, tag="big",
                                         name=f"pj_{b}_{qc}_{id(w_t)}")
                        for et in range(ET):
                            nc.tensor.matmul(
                                ps, w_t[:, et, :], xtile[:, et, :],
                                start=(et == 0), stop=(et == ET - 1),
                            )
                        nc.vector.tensor_scalar_add(
                            dst[:, qc * 512 : (qc + 1) * 512], ps, bias_t
                        )
                    return go

                def v_chunk(qc, vtt):
                    def go():
                        xtile = cell[qc]
                        tt = qc * 4 + vtt
                        pv = ps_big.tile([P, DSL], f32, tag="big",
                                         name=f"pv_{b}_{tt}")
                        for et in range(ET):
                            nc.tensor.matmul(
                                pv, xtile[:, et, vtt * P : (vtt + 1) * P],
                                wv[:, et, :],
                                start=(et == 0), stop=(et == ET - 1),
                            )
                        nc.vector.tensor_copy(vt[:, tt, 0:64], pv[:, 0:64])
                        nc.vector.tensor_copy(vt[:, tt, 65:129],
                                              pv[:, 64:128])
                    return go

                chunks = []
                for qc in range(4):
                    chunks.append(load_chunk(qc))
                    chunks.append(qk_chunk(qc, wq, bq, qT))
                    chunks.append(qk_chunk(qc, wk, bk, kT))
                    for vtt in range(4):
                        chunks.append(v_chunk(qc, vtt))
                return qT, kT, vt, chunks

            def outproj_chunks(b, oTs):
                """Output projection chunk closures for batch b."""
                chunks = []

                def one(br, tt):
                    def go():
                        po = ps_big.tile([P, D], f32, tag="big",
                                         name=f"po_{b}_{br}_{tt}")
                        for oc in range(2):
                            nc.tensor.matmul(
                                po[:, oc * 512 : (oc + 1) * 512],
                                oTs[br][:, tt * P : (tt + 1) * P],
                                wo[:, oc * 512 : (oc + 1) * 512],
                                start=True,
                                stop=True,
                            )
                        ob = outp.tile([P, D], bf16, tag="ob")
                        nc.scalar.copy(ob[:], po[:])
                        nc.sync.dma_start(
                            out_d[br, b, tt * P : (tt + 1) * P, :], ob[:]
                        )
                    return go

                for br in range(2):
                    for tt in range(TT):
                        chunks.append(one(br, tt))
                return chunks

            def attention(b, qT, kT, vt, oTs):
                for h in range(HPC):
                    hp = 64 * h
                    vlo, vhi = (0, 65) if h == 0 else (65, 130)
                    for qc in range(QC):
                        q0 = qc * QW
                        name = f"_{b}_{h}_{qc}"
                        exn = stash.tile([P, KT, QW], bf16, tag="exn",
                                         name=f"exn{name}")
                        acc = ps_acc.tile([P, QW], f32, tag="acc",
                                          name=f"accp{name}")
                        for kt in range(KT):
                            sc = ps_big.tile([P, QW], f32, tag="big",
                                             name=f"sc{name}_{kt}")
                            for fh in range(2):
                                nc.tensor.matmul(
                                    sc[:, fh * 512 : (fh + 1) * 512],
                                    kT[hp : hp + 64, kt * P : (kt + 1) * P],
                                    qT[hp : hp + 64,
                                       q0 + fh * 512 : q0 + (fh + 1) * 512],
                                    start=True,
                                    stop=True,
                                )
                            ex = expp.tile([P, QW], bf16, tag="ex")
                            nc.scalar.activation(ex, sc, Exp, scale=-0.125)
                            recip(exn[:, kt, :], ex)
                            for fh in range(2):
                                nc.tensor.matmul(
                                    acc[0:65, fh * 512 : (fh + 1) * 512],
                                    vt[:, kt, vlo:vhi],
                                    ex[:, fh * 512 : (fh + 1) * 512],
                                    start=(kt == 0),
                                    stop=(kt == KT - 1),
                                )
                            emit_pending_mms(kt, kt + 1)
                            if kt % 2 == 1:
                                drain_bg(1)
                        finish_pending()
                        normalize(acc, oTs[0], h, q0, name + "p")
                        accn = ps_acc.tile([P, QW], f32, tag="acc",
                                           name=f"accn{name}")
                        pending.update(exn=exn, acc=accn, vt=vt, vlo=vlo,
                                       vhi=vhi, oTs_br=oTs[1], h=h, q0=q0,
                                       name=name)

            # ---------- emission ----------
            qT0, kT0, vt0, pchunks0 = queue_projections(0)
            for ch in pchunks0:       # batch-0 projections run up front
                ch()
            qT1, kT1, vt1, pchunks1 = queue_projections(1)
            bg_queue.extend(pchunks1)  # batch-1 projections hide in b0 attn

            oTs0 = [otsp.tile([P, S], bf16, tag=f"oTs{br}", name=f"oTs{br}_0")
                    for br in range(2)]
            oTs1 = [otsp.tile([P, S], bf16, tag=f"oTs{br}", name=f"oTs{br}_1")
                    for br in range(2)]

            attention(0, qT0, kT0, vt0, oTs0)
            bg_queue.extend(outproj_chunks(0, oTs0))  # hide in b1 attn
            attention(1, qT1, kT1, vt1, oTs1)

            # tail: alternate the last pending PV with batch-1 br0
            # output-projection chunks; br1 chunks only after the pending
            # norm has written its oTs[1] slice (emission order is
            # semantics under Tile's tracing)
            op1 = outproj_chunks(1, oTs1)
            bg_queue.extend(op1[:TT])
            for kt in range(KT):
                emit_pending_mms(kt, kt + 1)
                drain_bg(1)
            finish_pending()
            bg_queue.extend(op1[TT:])
            drain_bg(len(bg_queue))
    _split_sync_waits(nc)
    # populate .instr bytes for extended instructions (InstCustomDveAnt);
    # raw Bass skips this pass and the NEFF compiler then sees an empty
    # instr -> "ISA wrong length"
    from concourse.library_overlay import lower_extended_insts

    lower_extended_insts(nc)
    return nc




def _get_nc():
    if "nc" not in _compiled:
        _compiled["nc"] = _build()
    return _compiled["nc"]


def _prep_in_maps(x, Wq, bq, Wk, bk, Wv, bv, Wo, bo):
    ET = D // P
    xf = np.ascontiguousarray(x.reshape(B * S, D))
    # x^T tiled: [p, et, token], e = et*128 + p
    xt = np.ascontiguousarray(
        xf.T.reshape(ET, P, B * S).transpose(1, 0, 2)
    ).astype(BF16)
    in_maps = []
    for c in range(NCORES):
        sl = slice(DSL * c, DSL * (c + 1))
        wqt = np.ascontiguousarray(
            Wq[sl].T.reshape(ET, P, DSL).transpose(1, 0, 2)
        ).astype(BF16)
        wkt = np.ascontiguousarray(
            Wk[sl].T.reshape(ET, P, DSL).transpose(1, 0, 2)
        ).astype(BF16)
        wvt = np.ascontiguousarray(
            Wv[sl].T.reshape(ET, P, DSL).transpose(1, 0, 2)
        ).astype(BF16)
        wot = np.ascontiguousarray(Wo[:, sl].T).astype(BF16)
        in_maps.append(
            {
                "xt": xt,
                "wq": wqt,
                "wk": wkt,
                "wv": wvt,
                "wo": wot,
                "bq": np.ascontiguousarray(bq[sl].reshape(P, 1)).astype(np.float32),
                "bk": np.ascontiguousarray(bk[sl].reshape(P, 1)).astype(np.float32),
            }
        )
    return in_maps


def kernel(x, Wq, bq, Wk, bk, Wv, bv, Wo, bo, _trace=False, _tmpdir=None):
    from concourse.bass_utils import run_bass_kernel_spmd

    x, Wq, bq, Wk, bk, Wv, bv, Wo, bo = (
        np.asarray(a, dtype=np.float32)
        for a in (x, Wq, bq, Wk, bk, Wv, bv, Wo, bo)
    )
    nc = _get_nc()
    in_maps = _prep_in_maps(x, Wq, bq, Wk, bk, Wv, bv, Wo, bo)
    res = run_bass_kernel_spmd(
        nc, in_maps, core_ids=list(range(NCORES)), trace=_trace, tmpdir=_tmpdir
    )
    total = np.zeros((2, B, S, D), np.float32)
    for c in range(NCORES):
        total += np.asarray(res.results[c]["out"], dtype=np.float32)
    const_vec = (bv @ Wo.T + bo).astype(np.float32)
    out = total[0] + const_vec
    out_comp = total[1] + const_vec
    if _trace:
        kernel._last_result = res
    return (out, out_comp)



# revision 13
# speedup vs baseline: 1.3273x; 1.0248x over previous
"""Trainium2 Bass kernel for nn_CausalAttention_5815385719336.

Dual-softmax attention: out = softmax(-QK^T/8) V Wo^T (+bias folds),
out_comp = softmax(+QK^T/8) V Wo^T.  B=2, S=2048, D=1024, H=16, DK=64.

Sharding (8 cores): Megatron-style head parallel.  Core c owns heads
(2c, 2c+1) = output dims [128c, 128c+128) of the QKV projections.  Each
core computes its head slice of Q/K/V for both batches, the full [S,S]
attention for its 4 (b, head) units (both softmax branches), and a
partial output projection o_slice @ Wo_slice^T.  The host sums the 8
partial outputs and adds the bias fold (bv @ Wo^T + bo).

On-device dataflow is fully "transposed": the host ships x^T (and W^T
slices) so every matmul contracts along partitions with zero on-device
transposes.  Scores are built as scores^T [k, q]; the negative-branch
exp runs on the scalar engine straight out of PSUM and the positive
branch is its reciprocal on the DVE (custom RECIPROCAL_APPROX_FAST op),
splitting the 33.6M-element exp workload across two engines.  P^T @ V
needs no transpose because P^T is exactly what the PV matmul wants as
its moving operand.  The softmax denominator comes for free from a
ones-column appended to V (one extra PSUM row per head), is
reciprocated on the DVE and broadcast across partitions by a
ones-matmul on the PE; PSUM->SBUF output casts run on the scalar
engine (closer to PSUM) to keep the DVE free for the reciprocals.
"""

import numpy as np
import ml_dtypes

B, S, D, H, DK = 2, 2048, 1024, 16, 64
NCORES = 8
HPC = H // NCORES          # heads per core = 2
DSL = HPC * DK             # d-slice per core = 128
P = 128
BF16 = ml_dtypes.bfloat16

_compiled = {}


def _install_drain_split():
    """walrus in this container rejects >1 sync wait on the Tile tail
    Drain; split extra waits into standalone wait_ge instructions."""
    import concourse.tile as tile
    from concourse.vector_clock import ScopedClock

    if getattr(tile.TileContext, "_drain_split_installed", False):
        return

    def _drain_and_barrier(self, tick_clock, wait_clock):
        nc = self.nc
        drain_inst = nc.sync.drain()
        wait_clock.add_sem_waits(
            drain_inst.ins, ScopedClock({None: tick_clock.global_clock})
        )
        si = drain_inst.ins.sync_info
        if si is not None and si.on_wait and len(si.on_wait) > 1:
            waits = list(si.on_wait)
            handles = {h.num: h for h in self.sems.allocated().values()}
            si.on_wait = waits[:1]
            for w in waits[1:]:
                assert w.wait_mode == "sem-ge-imm", w.wait_mode
                nc.sync.wait_ge(handles[w.id], w.wait_value)
        nc.all_engine_barrier()
        popped = nc._tile_sem_poison_stack.pop()
        assert popped is self._sem_poison
        nc.clear_and_free_semaphores(list(self.sems.allocated().values()))
        nc.all_engine_barrier()

    tile.TileContext._drain_and_barrier = _drain_and_barrier
    tile.TileContext._drain_split_installed = True


def _split_sync_waits(nc, max_waits=1):
    """walrus in this container has a small per-instruction sync-wait
    capacity.  Hoist excess waits onto standalone EventSemaphore
    instructions inserted just before the owner on the same engine —
    program order within an engine keeps the semantics identical."""
    from concourse import mybir

    n = 0
    for bb in nc.main_func.blocks:
        out = []
        for ins in bb.instructions:
            si = ins.sync_info
            if si is not None and si.on_wait and len(si.on_wait) > max_waits:
                waits = list(si.on_wait)
                for w in waits[:-max_waits]:
                    wi = mybir.InstEventSemaphore(name=f"W-split-{n}", ins=[], outs=[])
                    n += 1
                    wi.engine = ins.engine
                    wi.sync_info = mybir.SyncInfo(on_wait=[w], on_update=[])
                    out.append(wi)
                si.on_wait = waits[-max_waits:]
            out.append(ins)
        if n:
            bb.instructions = out


def _build():
    import concourse.bass as bass
    import concourse.tile as tile
    from concourse import mybir

    _install_drain_split()

    from concourse.dve_ops import (
        RECIP_APPROX_FAST_CONSTS,
        RECIPROCAL_APPROX_FAST,
    )

    f32 = mybir.dt.float32
    bf16 = mybir.dt.bfloat16
    Exp = mybir.ActivationFunctionType.Exp
    NT = B * S                      # 4096 tokens
    ET = D // P                     # 8 e-tiles
    RC = RECIP_APPROX_FAST_CONSTS

    nc = bass.Bass()
    xt_d = nc.declare_dram_parameter("xt", [P, ET, NT], bf16, isOutput=False)
    wq_d = nc.declare_dram_parameter("wq", [P, ET, DSL], bf16, isOutput=False)
    wk_d = nc.declare_dram_parameter("wk", [P, ET, DSL], bf16, isOutput=False)
    wv_d = nc.declare_dram_parameter("wv", [P, ET, DSL], bf16, isOutput=False)
    wo_d = nc.declare_dram_parameter("wo", [P, D], bf16, isOutput=False)
    bq_d = nc.declare_dram_parameter("bq", [P, 1], f32, isOutput=False)
    bk_d = nc.declare_dram_parameter("bk", [P, 1], f32, isOutput=False)
    out_d = nc.declare_dram_parameter("out", [2, B, S, D], bf16, isOutput=True)

    KT = S // P                     # 16 k-tiles per batch
    TT = S // P                     # 16 token-tiles per batch
    QC = 4                          # q chunks per batch (both heads at once)
    QW = S // QC                    # 512

    with tile.TileContext(nc) as tc:
        with (
            tc.tile_pool(name="singles", bufs=1) as singles,
            tc.tile_pool(name="xst", bufs=2) as xst,
            tc.tile_pool(name="perb", bufs=2) as perb,
            tc.tile_pool(name="stash", bufs=2) as stash,
            tc.tile_pool(name="expp", bufs=3) as expp,
            tc.tile_pool(name="otsp", bufs=2) as otsp,
            tc.tile_pool(name="normp", bufs=3) as normp,
            tc.tile_pool(name="outp", bufs=3) as outp,
            # 8 PSUM banks total: ps_big 2x[128,1024] = 4 (scores pair, denom
            # broadcast, projections, outproj), ps_pos 2x[128,512] = 2 (the
            # positive-branch PV accumulators of the current unit), ps_neg
            # 2x[128,512] = 2 (the deferred negative-branch accumulators).
            tc.tile_pool(name="ps_big", bufs=2, space="PSUM") as ps_big,
            tc.tile_pool(name="ps_pos", bufs=2, space="PSUM") as ps_pos,
            tc.tile_pool(name="ps_neg", bufs=2, space="PSUM") as ps_neg,
        ):
            wq = singles.tile([P, ET, DSL], bf16)
            nc.sync.dma_start(wq[:], wq_d[:])
            wk = singles.tile([P, ET, DSL], bf16)
            nc.sync.dma_start(wk[:], wk_d[:])
            wv = singles.tile([P, ET, DSL], bf16)
            nc.sync.dma_start(wv[:], wv_d[:])
            wo = singles.tile([P, D], bf16)
            nc.sync.dma_start(wo[:], wo_d[:])
            bq = singles.tile([P, 1], f32)
            nc.sync.dma_start(bq[:], bq_d[:])
            bk = singles.tile([P, 1], f32)
            nc.sync.dma_start(bk[:], bk_d[:])
            ones_sb = singles.tile([P, 64], bf16)
            nc.vector.memset(ones_sb[:], 1.0)

            def recip(out_ap, in_ap):
                """out = 1/in on the DVE (custom op, ~51 ULP in fp32; the
                bf16 write rounds it to bf16 anyway).  Frees the ACT engine
                from the positive-branch exp: exp(+s) = 1/exp(-s)."""
                nc.vector._custom_dve(
                    RECIPROCAL_APPROX_FAST, out=out_ap, in0=in_ap,
                    s0=RC["s0"], s1=RC["s1"], imm2=RC["imm2"],
                )

            # ---------- helpers ----------
            # background queue of deferred PE-work closures, drained at
            # attention kt-boundaries to keep the PE dense (HAM-warm) while
            # ACT paces the kernel
            bg_queue = []

            def drain_bg(n=1):
                for _ in range(n):
                    if not bg_queue:
                        return
                    bg_queue.pop(0)()

            # one pending negative-branch PV accumulation (both heads),
            # interleaved into the next unit's kt loop
            pending = {}

            def emit_pending_mms(k0, k1):
                if not pending:
                    return
                exn, accn, vtp = (pending[k] for k in ("exn", "accn", "vt"))
                for kt in range(k0, k1):
                    for h in range(2):
                        nc.tensor.matmul(
                            accn[h][0:65, :],
                            vtp[:, kt, 65 * h : 65 * h + 65],
                            exn[:, kt, h * QW : (h + 1) * QW],
                            start=(kt == 0),
                            stop=(kt == KT - 1),
                        )

            def normalize_pair(accs, oTs_br, q0, name):
                """Both heads of one (b, qc, branch): 1/denom via the DVE
                recip; broadcast via ones-matmul into a shared bc tile;
                multiply the unnormalized oT rows into their oTs slice."""
                bc = ps_big.tile([P, 1024], f32, tag="big", name=f"bc{name}")
                for h in range(2):
                    acc = accs[h]
                    hp = 64 * h
                    rcp = normp.tile([P, QW], bf16, tag="rcp",
                                     name=f"rcp{name}{h}")
                    # the custom-DVE op only functions at base_partition 0,
                    # so reciprocate rows 0..64 (cost is free-dim-bound; the
                    # junk recips of the o-rows are never read — only the
                    # denominator row 64 feeds the broadcast matmul)
                    recip(rcp[0:65, :], acc[0:65, :])
                    oTu = normp.tile([P, QW], bf16, tag="oTu",
                                     name=f"oTu{name}{h}")
                    nc.scalar.copy(oTu[0:64, :], acc[0:64, :])
                    if h == 1:
                        # shift head-1 rows to partitions 64:128 so the
                        # output projection contracts both heads at once
                        oTu2 = normp.tile([P, QW], bf16, tag="oTu2",
                                          name=f"oTu2{name}")
                        nc.sync.dma_start(oTu2[64:128, :], oTu[0:64, :])
                        oTu = oTu2
                    bc_ap = (bc[0:64, 0:QW] if h == 0
                             else bc[64:128, QW : 2 * QW])
                    nc.tensor.matmul(bc_ap, ones_sb[64:65, :], rcp[64:65, :],
                                     start=True, stop=True)
                    nc.vector.tensor_mul(
                        oTs_br[hp : hp + 64, q0 : q0 + QW],
                        oTu[hp : hp + 64, :],
                        bc_ap,
                    )

            def finish_pending():
                if not pending:
                    return
                normalize_pair(pending["accn"], pending["oTs_br"],
                               pending["q0"], pending["name"] + "n")
                pending.clear()

            def queue_projections(b):
                """Queue Q/K/V projection chunks for batch b (streamed x)."""
                t0 = b * S
                qT = perb.tile([P, S], bf16, tag="qT", name=f"qT_{b}")
                kT = perb.tile([P, S], bf16, tag="kT", name=f"kT_{b}")
                vt = perb.tile([P, TT, 130], bf16, tag="vt", name=f"vt_{b}")
                nc.vector.memset(vt[:, :, 64], 1.0)
                nc.vector.memset(vt[:, :, 129], 1.0)
                cell = {}

                def load_chunk(qc):
                    def go():
                        xtile = xst.tile([P, ET, 512], bf16, tag="xtile",
                                         name=f"xt_{b}_{qc}")
                        nc.sync.dma_start(
                            xtile[:],
                            xt_d[:, :, t0 + qc * 512 : t0 + (qc + 1) * 512],
                        )
                        cell[qc] = xtile
                    return go

                def qk_chunk(qc, w_t, bias_t, dst):
                    def go():
                        xtile = cell[qc]
                        ps = ps_big.tile([P, 512], f32, tag="big",
                                         name=f"pj_{b}_{qc}_{id(w_t)}")
                        for et in range(ET):
                            nc.tensor.matmul(
                                ps, w_t[:, et, :], xtile[:, et, :],
                                start=(et == 0), stop=(et == ET - 1),
                            )
                        nc.vector.tensor_scalar_add(
                            dst[:, qc * 512 : (qc + 1) * 512], ps, bias_t
                        )
                    return go

                def v_chunk(qc, vtt):
                    def go():
                        xtile = cell[qc]
                        tt = qc * 4 + vtt
                        pv = ps_big.tile([P, DSL], f32, tag="big",
                                         name=f"pv_{b}_{tt}")
                        for et in range(ET):
                            nc.tensor.matmul(
                                pv, xtile[:, et, vtt * P : (vtt + 1) * P],
                                wv[:, et, :],
                                start=(et == 0), stop=(et == ET - 1),
                            )
                        nc.vector.tensor_copy(vt[:, tt, 0:64], pv[:, 0:64])
                        nc.vector.tensor_copy(vt[:, tt, 65:129],
                                              pv[:, 64:128])
                    return go

                chunks = []
                for qc in range(4):
                    chunks.append(load_chunk(qc))
                    chunks.append(qk_chunk(qc, wq, bq, qT))
                    chunks.append(qk_chunk(qc, wk, bk, kT))
                    for vtt in range(4):
                        chunks.append(v_chunk(qc, vtt))
                return qT, kT, vt, chunks

            def outproj_chunks(b, oTs):
                """Output projection chunk closures for batch b."""
                chunks = []

                def one(br, tt):
                    def go():
                        po = ps_big.tile([P, D], f32, tag="big",
                                         name=f"po_{b}_{br}_{tt}")
                        for oc in range(2):
                            nc.tensor.matmul(
                                po[:, oc * 512 : (oc + 1) * 512],
                                oTs[br][:, tt * P : (tt + 1) * P],
                                wo[:, oc * 512 : (oc + 1) * 512],
                                start=True,
                                stop=True,
                            )
                        ob = outp.tile([P, D], bf16, tag="ob")
                        nc.scalar.copy(ob[:], po[:])
                        nc.sync.dma_start(
                            out_d[br, b, tt * P : (tt + 1) * P, :], ob[:]
                        )
                    return go

                for br in range(2):
                    for tt in range(TT):
                        chunks.append(one(br, tt))
                return chunks

            def attention(b, qT, kT, vt, oTs):
                """Both heads together per (qc, kt): the two DK=64 score
                matmuls are row-tiled into opposite halves of the PE array
                (concurrent), writing the two banks of one [128,1024] PSUM
                tile, so a single exp / recip covers both heads.  Emission
                is software-pipelined for the in-order PE queue: scores of
                kt+1 and the deferred negative-branch PVs go ahead of the
                exp-dependent positive PVs of kt."""
                def qk(qc, kt):
                    q0 = qc * QW
                    sc = ps_big.tile([P, 1024], f32, tag="big",
                                     name=f"sc_{b}_{qc}_{kt}")
                    for h in range(2):
                        nc.tensor.matmul(
                            sc[:, h * QW : (h + 1) * QW],
                            kT[64 * h : 64 * h + 64, kt * P : (kt + 1) * P],
                            qT[64 * h : 64 * h + 64, q0 : q0 + QW],
                            start=True,
                            stop=True,
                            tile_position=(64 * h, 0),
                        )
                    return sc

                for qc in range(QC):
                    q0 = qc * QW
                    name = f"_{b}_{qc}"
                    exn = stash.tile([P, KT, 1024], bf16, tag="exn",
                                     name=f"exn{name}")
                    acc = [ps_pos.tile([P, QW], f32, tag="acc",
                                       name=f"acc{name}_{h}")
                           for h in range(2)]
                    sc = qk(qc, 0)
                    for kt in range(KT):
                        sc_next = qk(qc, kt + 1) if kt + 1 < KT else None
                        ex = expp.tile([P, 1024], bf16, tag="ex")
                        nc.scalar.activation(ex, sc, Exp, scale=-0.125)
                        recip(exn[:, kt, :], ex)
                        emit_pending_mms(kt, kt + 1)
                        for h in range(2):
                            nc.tensor.matmul(
                                acc[h][0:65, :],
                                vt[:, kt, 65 * h : 65 * h + 65],
                                ex[:, h * QW : (h + 1) * QW],
                                start=(kt == 0),
                                stop=(kt == KT - 1),
                            )
                        if kt % 2 == 1:
                            drain_bg(1)
                        sc = sc_next
                    finish_pending()
                    normalize_pair(acc, oTs[0], q0, name + "p")
                    accn = [ps_neg.tile([P, QW], f32, tag="accn",
                                        name=f"accn{name}_{h}")
                            for h in range(2)]
                    pending.update(exn=exn, accn=accn, vt=vt,
                                   oTs_br=oTs[1], q0=q0, name=name)

            # ---------- emission ----------
            qT0, kT0, vt0, pchunks0 = queue_projections(0)
            for ch in pchunks0:       # batch-0 projections run up front
                ch()
            qT1, kT1, vt1, pchunks1 = queue_projections(1)
            bg_queue.extend(pchunks1)  # batch-1 projections hide in b0 attn

            oTs0 = [otsp.tile([P, S], bf16, tag=f"oTs{br}", name=f"oTs{br}_0")
                    for br in range(2)]
            oTs1 = [otsp.tile([P, S], bf16, tag=f"oTs{br}", name=f"oTs{br}_1")
                    for br in range(2)]

            attention(0, qT0, kT0, vt0, oTs0)
            bg_queue.extend(outproj_chunks(0, oTs0))  # hide in b1 attn
            attention(1, qT1, kT1, vt1, oTs1)

            # tail: alternate the last pending PV with batch-1 br0
            # output-projection chunks; br1 chunks only after the pending
            # norm has written its oTs[1] slice (emission order is
            # semantics under Tile's tracing)
            op1 = outproj_chunks(1, oTs1)
            bg_queue.extend(op1[:TT])
            for kt in range(KT):
                emit_pending_mms(kt, kt + 1)
                drain_bg(1)
            finish_pending()
            bg_queue.extend(op1[TT:])
            drain_bg(len(bg_queue))
    _split_sync_waits(nc)
    # populate .instr bytes for extended instructions (InstCustomDveAnt);
    # raw Bass skips this pass and the NEFF compiler then sees an empty
    # instr -> "ISA wrong length"
    from concourse.library_overlay import lower_extended_insts

    lower_extended_insts(nc)
    return nc




def _get_nc():
    if "nc" not in _compiled:
        _compiled["nc"] = _build()
    return _compiled["nc"]


def _prep_in_maps(x, Wq, bq, Wk, bk, Wv, bv, Wo, bo):
    ET = D // P
    xf = np.ascontiguousarray(x.reshape(B * S, D))
    # x^T tiled: [p, et, token], e = et*128 + p
    xt = np.ascontiguousarray(
        xf.T.reshape(ET, P, B * S).transpose(1, 0, 2)
    ).astype(BF16)
    in_maps = []
    for c in range(NCORES):
        sl = slice(DSL * c, DSL * (c + 1))
        wqt = np.ascontiguousarray(
            Wq[sl].T.reshape(ET, P, DSL).transpose(1, 0, 2)
        ).astype(BF16)
        wkt = np.ascontiguousarray(
            Wk[sl].T.reshape(ET, P, DSL).transpose(1, 0, 2)
        ).astype(BF16)
        wvt = np.ascontiguousarray(
            Wv[sl].T.reshape(ET, P, DSL).transpose(1, 0, 2)
        ).astype(BF16)
        wot = np.ascontiguousarray(Wo[:, sl].T).astype(BF16)
        in_maps.append(
            {
                "xt": xt,
                "wq": wqt,
                "wk": wkt,
                "wv": wvt,
                "wo": wot,
                "bq": np.ascontiguousarray(bq[sl].reshape(P, 1)).astype(np.float32),
                "bk": np.ascontiguousarray(bk[sl].reshape(P, 1)).astype(np.float32),
            }
        )
    return in_maps


def kernel(x, Wq, bq, Wk, bk, Wv, bv, Wo, bo, _trace=False, _tmpdir=None):
    from concourse.bass_utils import run_bass_kernel_spmd

    x, Wq, bq, Wk, bk, Wv, bv, Wo, bo = (
        np.asarray(a, dtype=np.float32)
        for a in (x, Wq, bq, Wk, bk, Wv, bv, Wo, bo)
    )
    nc = _get_nc()
    in_maps = _prep_in_maps(x, Wq, bq, Wk, bk, Wv, bv, Wo, bo)
    res = run_bass_kernel_spmd(
        nc, in_maps, core_ids=list(range(NCORES)), trace=_trace, tmpdir=_tmpdir
    )
    total = np.zeros((2, B, S, D), np.float32)
    for c in range(NCORES):
        total += np.asarray(res.results[c]["out"], dtype=np.float32)
    const_vec = (bv @ Wo.T + bo).astype(np.float32)
    out = total[0] + const_vec
    out_comp = total[1] + const_vec
    if _trace:
        kernel._last_result = res
    return (out, out_comp)

